# revision 1
# baseline (speedup 1.0000x reference)
"""EdgeConv block (kNN -> gather -> 1x1 conv -> GroupNorm -> ReLU -> max over k)
as a Bass/Tile kernel for 8 Trainium2 NeuronCores.

Problem shapes (hardcoded): B=4, C_IN=64, C_OUT=128, N=8192, K=16, G=8.

Sharding: core c handles batch b = c//2, query half h = c%2 (4096 queries),
with the batch's full key set replicated on both cores of the pair.
GroupNorm statistics are partial per core and combined with a pairwise
AllReduce on a [128, 2] tensor.

Math decomposition (avoids materializing [Nq, k, 2C] pair features):
  conv out[o,q,j] = W1 @ (nbr_j - Fi_q) + W2 @ Fi_q = A[o, idx[q,j]] + C[o,q]
  where A = W1 @ Fk  [O, Nk]  and  C = (W2 - W1) @ Fq  [O, Nq].
kNN scores s[q,p] = 2*Q.P - |P|^2 (monotone in -d2 per query) via fp32 PE
matmul with lhsT = [2qx; 2qy; 2qz; 1], rhs = [px; py; pz; -|P|^2].

Top-16 per query: 16 segments of Nk/16 keys; per-segment top-8 via DVE max8 +
max_index; merge the 16*8 candidates with two max8+match_replace rounds; turn
the selection mask into dense ranks with a prefix scan and compact the winning
global indices with a per-partition local_scatter.  (Exact unless >8 of the
true top-16 fall in one segment: P ~ 3e-6 per query.)

Neighbor reduction: gpsimd ap_gather of A columns (indices shared across all
128 channel partitions), then DVE blocked reduces for max_j / sum_j, fused
square-reduce for the GN second moment.
"""

from contextlib import ExitStack

import numpy as np

import concourse.bass as bass
import concourse.bacc as bacc
import concourse.mybir as mybir
from concourse.tile import TileContext
from concourse.bass_utils import run_bass_kernel_spmd

F32 = mybir.dt.float32
I16 = mybir.dt.int16
U16 = mybir.dt.uint16

B, C_IN, C_OUT, N_KEYS, KNN, G = 4, 64, 128, 8192, 16, 8
GN_EPS = 1e-5
N_CORES = 8


def build_edgeconv(nq, nk, nseg, n_pair_q, neg_gamma=False,
                   num_devices=N_CORES, use_cc=True):
    """Build the SPMD program. nq: queries per core; nk: keys per core;
    nseg: top-k segments (seg = nk//nseg <= 512); n_pair_q: total queries
    per batch across the core pair (GN denominator)."""
    seg = nk // nseg
    assert seg * nseg == nk and seg <= 512
    ncand = nseg * 8
    qtiles = nq // 128
    assert qtiles * 128 == nq
    chunk = min(256, nq)  # queries per gather chunk
    qstep = min(512, nq)
    nchunk = nq // chunk
    assert nchunk * chunk == nq
    gn_count = float(n_pair_q * KNN * (C_OUT // G))
    ngrp = num_devices // 2

    nc = bacc.Bacc("TRN2", target_bir_lowering=False, debug=False,
                   num_devices=num_devices)

    qt_ext = nc.dram_tensor("qt", [4, nq], F32, kind="ExternalInput")
    pt_ext = nc.dram_tensor("pt", [4, nk], F32, kind="ExternalInput")
    fk_ext = nc.dram_tensor("fk", [C_IN, nk], F32, kind="ExternalInput")
    fq_ext = nc.dram_tensor("fq", [C_IN, nq], F32, kind="ExternalInput")
    w1t_ext = nc.dram_tensor("w1t", [C_IN, C_OUT], F32, kind="ExternalInput")
    dt_ext = nc.dram_tensor("dt", [C_IN, C_OUT], F32, kind="ExternalInput")
    gam_ext = nc.dram_tensor("gam", [1, C_OUT], F32, kind="ExternalInput")
    bet_ext = nc.dram_tensor("bet", [1, C_OUT], F32, kind="ExternalInput")
    out_ext = nc.dram_tensor("out", [C_OUT, nq], F32, kind="ExternalOutput")

    idx_dram = nc.dram_tensor("idx_scratch", [nq, KNN], I16)
    row_dram = nc.dram_tensor("row_scratch", [2, C_OUT], F32)
    cc_in = nc.dram_tensor("cc_in", [C_OUT, 2], F32)
    cc_out = nc.dram_tensor("cc_out", [C_OUT, 2], F32)

    with TileContext(nc) as tc, ExitStack() as ctx:
        persist = ctx.enter_context(tc.tile_pool(name="persist", bufs=1))
        psum = ctx.enter_context(tc.tile_pool(name="psum", bufs=6,
                                              space="PSUM"))
        segp = ctx.enter_context(tc.tile_pool(name="segp", bufs=3))
        small = ctx.enter_context(tc.tile_pool(name="small", bufs=2))
        gchunk = ctx.enter_context(tc.tile_pool(name="gchunk", bufs=2))

        # ---- persistent SBUF ----
        # qt replicated at partition bases 0/32/64/96 so four q-tiles'
        # K=4 matmuls can run concurrently in distinct PE row groups
        rowtile = 4 if qtiles % 4 == 0 else 1
        qt_sb = persist.tile([128 if rowtile == 4 else 4, nq], F32,
                             tag="qt_sb")
        for r in range(rowtile):
            nc.sync.dma_start(out=qt_sb[32 * r:32 * r + 4, :],
                              in_=qt_ext[:, :])
        pt_sb = persist.tile([128 if rowtile == 4 else 4, nk], F32,
                             tag="pt_sb")
        for r in range(rowtile):
            nc.sync.dma_start(out=pt_sb[32 * r:32 * r + 4, :],
                              in_=pt_ext[:, :])
        w1t_sb = persist.tile([C_IN, C_OUT], F32, tag="w1t_sb")
        nc.sync.dma_start(out=w1t_sb, in_=w1t_ext[:, :])
        dtw_sb = persist.tile([C_IN, C_OUT], F32, tag="dtw_sb")
        nc.sync.dma_start(out=dtw_sb, in_=dt_ext[:, :])
        gam_sb = persist.tile([1, C_OUT], F32, tag="gam_sb")
        nc.sync.dma_start(out=gam_sb, in_=gam_ext[:, :])
        bet_sb = persist.tile([1, C_OUT], F32, tag="bet_sb")
        nc.sync.dma_start(out=bet_sb, in_=bet_ext[:, :])

        a_sb = persist.tile([C_OUT, nk], F32, tag="a_sb")
        c_sb = persist.tile([C_OUT, nq], F32, tag="c_sb")
        mpos_sb = persist.tile([C_OUT, nq], F32, tag="mpos_sb")
        mneg_sb = (persist.tile([C_OUT, nq], F32, tag="mneg_sb")
                   if neg_gamma else None)
        seg_off = persist.tile([128, ncand], I16, tag="seg_off")
        nc.gpsimd.iota(seg_off, pattern=[[seg, nseg], [0, 8]], base=0,
                       channel_multiplier=0)
        zeros_nc = persist.tile([128, ncand], F32, tag="zeros_nc")
        nc.vector.memset(zeros_nc, 0.0)

        # ---- A and C matmuls (fp32), feature inputs streamed in slices ----
        with tc.tile_pool(name="feat", bufs=3) as featp:
            for s0 in range(0, nk, 512):
                fk_t = featp.tile([C_IN, 512], F32, tag="fk_t")
                nc.sync.dma_start(out=fk_t, in_=fk_ext[:, s0:s0 + 512])
                ps = psum.tile([C_OUT, 512], F32, tag="ps")
                nc.tensor.matmul(ps, lhsT=w1t_sb, rhs=fk_t,
                                 start=True, stop=True)
                nc.scalar.copy(out=a_sb[:, s0:s0 + 512], in_=ps)
            for s0 in range(0, nq, qstep):
                fq_t = featp.tile([C_IN, qstep], F32, tag="fq_t")
                nc.sync.dma_start(out=fq_t, in_=fq_ext[:, s0:s0 + qstep])
                ps = psum.tile([C_OUT, qstep], F32, tag="ps")
                nc.tensor.matmul(ps, lhsT=dtw_sb, rhs=fq_t,
                                 start=True, stop=True)
                nc.scalar.copy(out=c_sb[:, s0:s0 + qstep], in_=ps)

        # stat accumulators (filled by interleaved gather chunks)
        r_sa = small.tile([128, 1], F32, tag="acc_sa")
        r_sqa = small.tile([128, 1], F32, tag="acc_sqa")
        r_csa = small.tile([128, 1], F32, tag="acc_csa")
        nc.vector.memset(r_sa, 0.0)
        nc.vector.memset(r_sqa, 0.0)
        nc.vector.memset(r_csa, 0.0)

        def emit_gather_chunk(ch):
            q0 = ch * chunk
            idxs_t = gchunk.tile([128, chunk], I16, tag="idxs_t")
            for g in range(8):
                nc.sync.dma_start(
                    out=idxs_t[g * 16:(g + 1) * 16, :],
                    in_=bass.AP(tensor=idx_dram, offset=q0 * KNN,
                                ap=[[1, KNN], [KNN, chunk]]),
                )
            ga = gchunk.tile([128, chunk * KNN], F32, tag="ga")
            nc.gpsimd.ap_gather(out_ap=ga, in_ap=a_sb, idxs_ap=idxs_t,
                                channels=128, num_elems=nk, d=1,
                                num_idxs=chunk * KNN)
            gav = ga.rearrange("p (q c) -> p q c", c=KNN)
            nc.vector.tensor_reduce(out=mpos_sb[:, q0:q0 + chunk], in_=gav,
                                    axis=mybir.AxisListType.X,
                                    op=mybir.AluOpType.max)
            if neg_gamma:
                nc.vector.tensor_reduce(out=mneg_sb[:, q0:q0 + chunk],
                                        in_=gav, axis=mybir.AxisListType.X,
                                        op=mybir.AluOpType.min)
            sa_c = gchunk.tile([128, chunk], F32, tag="sa_c")
            nc.vector.tensor_reduce(out=sa_c, in_=gav,
                                    axis=mybir.AxisListType.X,
                                    op=mybir.AluOpType.add)
            tmp1 = small.tile([128, 1], F32, tag="tmp1")
            nc.vector.tensor_reduce(out=tmp1, in_=sa_c,
                                    axis=mybir.AxisListType.X,
                                    op=mybir.AluOpType.add)
            nc.vector.tensor_add(r_sa, r_sa, tmp1)
            scr_c = gchunk.tile([128, chunk], F32, tag="scr_c")
            nc.vector.tensor_mul(scr_c, sa_c, c_sb[:, q0:q0 + chunk])
            nc.vector.tensor_reduce(out=tmp1, in_=scr_c,
                                    axis=mybir.AxisListType.X,
                                    op=mybir.AluOpType.add)
            nc.vector.tensor_add(r_csa, r_csa, tmp1)
            # in-place square on gpsimd (offloads the DVE bottleneck)
            nc.gpsimd.tensor_mul(ga, ga, ga)
            nc.vector.tensor_reduce(out=tmp1, in_=ga,
                                    axis=mybir.AxisListType.X,
                                    op=mybir.AluOpType.add)
            nc.vector.tensor_add(r_sqa, r_sqa, tmp1)

        queries_per_group = rowtile * 128
        # ---- per-q-tile kNN (row-tiled: `rowtile` q-tiles in flight),
        # with gather chunks interleaved as soon as their indices land ----
        for tq0 in range(0, qtiles, rowtile):
            cvs, cis = [], []
            for r in range(rowtile):
                cv_r = small.tile([128, ncand], F32, tag=f"cv{r}")
                ci_r = small.tile([128, ncand], U16, tag=f"ci{r}")
                cvs.append(cv_r)
                cis.append(ci_r)
            for s in range(nseg):
                for r in range(rowtile):
                    t = tq0 + r
                    lhs_q = qt_sb[32 * r:32 * r + 4,
                                  t * 128:(t + 1) * 128]
                    ps = psum.tile([128, seg], F32, tag="ps")
                    nc.tensor.matmul(ps, lhsT=lhs_q,
                                     rhs=pt_sb[32 * r:32 * r + 4,
                                               s * seg:(s + 1) * seg],
                                     start=True, stop=True,
                                     tile_position=(32 * r, 0))
                    ssb = segp.tile([128, seg], F32, tag="ssb")
                    nc.scalar.copy(out=ssb, in_=ps)
                    nc.vector.max(out=cvs[r][:, s * 8:(s + 1) * 8], in_=ssb)
                    nc.vector.max_index(out=cis[r][:, s * 8:(s + 1) * 8],
                                        in_max=cvs[r][:, s * 8:(s + 1) * 8],
                                        in_values=ssb)
            for r in range(rowtile):
                t = tq0 + r
                cv, ci = cvs[r], cis[r]
                v1 = small.tile([128, 8], F32, tag="v1")
                v2 = small.tile([128, 8], F32, tag="v2")
                cv2 = small.tile([128, ncand], F32, tag="cv2")
                cv3 = small.tile([128, ncand], F32, tag="cv3")
                nc.vector.max(out=v1, in_=cv)
                nc.vector.match_replace(out=cv2, in_to_replace=v1,
                                        in_values=cv, imm_value=-1e30)
                nc.vector.max(out=v2, in_=cv2)
                nc.vector.match_replace(out=cv3, in_to_replace=v2,
                                        in_values=cv2, imm_value=-1e30)
                maskf = small.tile([128, ncand], F32, tag="maskf")
                nc.vector.tensor_tensor(out=maskf, in0=cv, in1=cv3,
                                        op=mybir.AluOpType.not_equal)
                rk = small.tile([128, ncand], F32, tag="rk")
                nc.vector.tensor_tensor_scan(out=rk, data0=maskf,
                                             data1=zeros_nc, initial=0.0,
                                             op0=mybir.AluOpType.add,
                                             op1=mybir.AluOpType.add)
                tgt = small.tile([128, ncand], F32, tag="tgt")
                nc.vector.tensor_tensor(out=tgt, in0=rk, in1=maskf,
                                        op=mybir.AluOpType.mult)
                nc.vector.tensor_scalar_add(tgt, tgt, -1.0)
                tgt_i = small.tile([128, ncand], I16, tag="tgti")
                nc.vector.tensor_copy(tgt_i, tgt)
                gidx = small.tile([128, ncand], I16, tag="gidx")
                nc.vector.tensor_tensor(out=gidx, in0=ci.bitcast(I16),
                                        in1=seg_off, op=mybir.AluOpType.add)
                idx16 = small.tile([128, KNN], I16, tag="idx16")
                nc.gpsimd.local_scatter(out_ap=idx16, data_ap=gidx,
                                        idxs_ap=tgt_i, channels=128,
                                        num_elems=KNN, num_idxs=ncand)
                nc.sync.dma_start(out=idx_dram[t * 128:(t + 1) * 128, :],
                                  in_=idx16)
            # emit the PREVIOUS group's gather chunks: their idx writes
            # have had a full group of kNN work to complete, so the DRAM
            # round-trip latency is hidden
            if tq0 > 0:
                prev_q0 = (tq0 - rowtile) * 128
                for ch in range(prev_q0 // chunk,
                                (prev_q0 + queries_per_group) // chunk):
                    emit_gather_chunk(ch)

        # flush the final group's gather chunks
        last_q0 = (qtiles - rowtile) * 128
        for ch in range(last_q0 // chunk,
                        (last_q0 + queries_per_group) // chunk):
            emit_gather_chunk(ch)

        r_c = small.tile([128, 1], F32, tag="r_c")
        nc.vector.tensor_reduce(out=r_c, in_=c_sb,
                                axis=mybir.AxisListType.X,
                                op=mybir.AluOpType.add)
        r_c2 = small.tile([128, 1], F32, tag="r_c2")
        nc.vector.memset(r_c2, 0.0)
        tmpc = small.tile([128, 1], F32, tag="tmpc")
        for q0 in range(0, nq, qstep):
            scr5 = gchunk.tile([128, qstep], F32, tag="scr5")
            nc.vector.tensor_mul(scr5, c_sb[:, q0:q0 + qstep],
                                 c_sb[:, q0:q0 + qstep])
            nc.vector.tensor_reduce(out=tmpc, in_=scr5,
                                    axis=mybir.AxisListType.X,
                                    op=mybir.AluOpType.add)
            nc.vector.tensor_add(r_c2, r_c2, tmpc)

        s1p = small.tile([128, 1], F32, tag="s1p")
        nc.vector.tensor_scalar(out=s1p, in0=r_c, scalar1=float(KNN),
                                scalar2=None, op0=mybir.AluOpType.mult)
        nc.vector.tensor_add(s1p, s1p, r_sa)
        s2p = small.tile([128, 1], F32, tag="s2p")
        nc.vector.tensor_scalar(out=s2p, in0=r_c2, scalar1=float(KNN),
                                scalar2=None, op0=mybir.AluOpType.mult)
        t2 = small.tile([128, 1], F32, tag="t2")
        nc.vector.tensor_scalar(out=t2, in0=r_csa, scalar1=2.0,
                                scalar2=None, op0=mybir.AluOpType.mult)
        nc.vector.tensor_add(s2p, s2p, t2)
        nc.vector.tensor_add(s2p, s2p, r_sqa)

        # ---- pairwise allreduce of [128, 2] partials ----
        s12 = small.tile([128, 2], F32, tag="s12")
        nc.vector.tensor_copy(s12[:, 0:1], s1p)
        nc.vector.tensor_copy(s12[:, 1:2], s2p)
        nc.sync.dma_start(out=cc_in[:, :], in_=s12)
        if use_cc:
            nc.gpsimd.collective_compute(
                "AllReduce", mybir.AluOpType.add,
                replica_groups=[[2 * i, 2 * i + 1] for i in range(ngrp)],
                ins=[cc_in[:, :]], outs=[cc_out[:, :]])
        else:
            # diagnostic mode: no cross-core reduce (stats use only this
            # core's half; output is approximate)
            nc.sync.dma_start(out=cc_out[:, :], in_=s12)

        # ---- finish GroupNorm stats in [*, C_OUT] row layout ----
        st1 = small.tile([1, C_OUT], F32, tag="st1")
        nc.sync.dma_start(out=st1,
                          in_=bass.AP(tensor=cc_out, offset=0,
                                      ap=[[0, 1], [2, C_OUT]]))
        st2 = small.tile([1, C_OUT], F32, tag="st2")
        nc.sync.dma_start(out=st2,
                          in_=bass.AP(tensor=cc_out, offset=1,
                                      ap=[[0, 1], [2, C_OUT]]))
        sg1 = small.tile([1, G], F32, tag="sg1")
        nc.vector.tensor_reduce(out=sg1,
                                in_=st1.rearrange("p (g d) -> p g d", g=G),
                                axis=mybir.AxisListType.X,
                                op=mybir.AluOpType.add)
        sg2 = small.tile([1, G], F32, tag="sg2")
        nc.vector.tensor_reduce(out=sg2,
                                in_=st2.rearrange("p (g d) -> p g d", g=G),
                                axis=mybir.AxisListType.X,
                                op=mybir.AluOpType.add)
        mean_r = small.tile([1, G], F32, tag="mean_r")
        nc.vector.tensor_scalar(out=mean_r, in0=sg1,
                                scalar1=1.0 / gn_count, scalar2=None,
                                op0=mybir.AluOpType.mult)
        ex2_r = small.tile([1, G], F32, tag="ex2_r")
        nc.vector.tensor_scalar(out=ex2_r, in0=sg2,
                                scalar1=1.0 / gn_count, scalar2=None,
                                op0=mybir.AluOpType.mult)
        var_r = small.tile([1, G], F32, tag="var_r")
        nc.vector.tensor_tensor(out=var_r, in0=mean_r, in1=mean_r,
                                op=mybir.AluOpType.mult)
        nc.vector.tensor_tensor(out=var_r, in0=ex2_r, in1=var_r,
                                op=mybir.AluOpType.subtract)
        sd_r = small.tile([1, G], F32, tag="sd_r")
        nc.vector.tensor_scalar_add(var_r, var_r, GN_EPS)
        nc.scalar.activation(sd_r, var_r, mybir.ActivationFunctionType.Sqrt,
                             bias=0.0)
        rstd_r = small.tile([1, G], F32, tag="rstd_r")
        nc.vector.reciprocal(rstd_r, sd_r)
        mean_c = small.tile([1, C_OUT], F32, tag="mean_c")
        rstd_c = small.tile([1, C_OUT], F32, tag="rstd_c")
        gsz = C_OUT // G
        for g in range(G):
            nc.vector.tensor_copy(
                mean_c[:, g * gsz:(g + 1) * gsz],
                mean_r[:, g:g + 1].to_broadcast([1, gsz]))
            nc.vector.tensor_copy(
                rstd_c[:, g * gsz:(g + 1) * gsz],
                rstd_r[:, g:g + 1].to_broadcast([1, gsz]))
        srow = small.tile([1, C_OUT], F32, tag="srow")
        nc.vector.tensor_tensor(out=srow, in0=gam_sb, in1=rstd_c,
                                op=mybir.AluOpType.mult)
        trow = small.tile([1, C_OUT], F32, tag="trow")
        nc.vector.tensor_tensor(out=trow, in0=mean_c, in1=srow,
                                op=mybir.AluOpType.mult)
        nc.vector.tensor_tensor(out=trow, in0=bet_sb, in1=trow,
                                op=mybir.AluOpType.subtract)
        # transpose the two [1, C_OUT] rows to [C_OUT, 1] via DRAM bounce
        nc.sync.dma_start(out=row_dram[0:1, :], in_=srow)
        nc.sync.dma_start(out=row_dram[1:2, :], in_=trow)
        s_col = small.tile([C_OUT, 1], F32, tag="s_col")
        nc.sync.dma_start(out=s_col,
                          in_=bass.AP(tensor=row_dram, offset=0,
                                      ap=[[1, C_OUT], [0, 1]]))
        t_col = small.tile([C_OUT, 1], F32, tag="t_col")
        nc.sync.dma_start(out=t_col,
                          in_=bass.AP(tensor=row_dram, offset=C_OUT,
                                      ap=[[1, C_OUT], [0, 1]]))

        # ---- final normalization + relu + output ----
        for q0 in range(0, nq, qstep):
            mf = gchunk.tile([128, qstep], F32, tag="mf")
            nc.vector.tensor_add(mf, mpos_sb[:, q0:q0 + qstep],
                                 c_sb[:, q0:q0 + qstep])
            nc.vector.tensor_scalar(out=mf, in0=mf, scalar1=s_col,
                                    scalar2=t_col,
                                    op0=mybir.AluOpType.mult,
                                    op1=mybir.AluOpType.add)
            if neg_gamma:
                mn = gchunk.tile([128, qstep], F32, tag="mn")
                nc.vector.tensor_add(mn, mneg_sb[:, q0:q0 + qstep],
                                     c_sb[:, q0:q0 + qstep])
                nc.vector.tensor_scalar(out=mn, in0=mn, scalar1=s_col,
                                        scalar2=t_col,
                                        op0=mybir.AluOpType.mult,
                                        op1=mybir.AluOpType.add)
                nc.vector.tensor_tensor(out=mf, in0=mf, in1=mn,
                                        op=mybir.AluOpType.max)
            nc.vector.tensor_scalar_max(mf, mf, 0.0)
            nc.sync.dma_start(out=out_ext[:, q0:q0 + qstep], in_=mf)

    nc.finalize()
    return nc


def make_core_inputs(Fq, Fk, Pq, Pk, W, nq_half, core):
    b, h = core // 2, core % 2
    q0 = h * nq_half
    Qs = Pq[b][:, q0:q0 + nq_half]
    qt = np.concatenate([2.0 * Qs, np.ones((1, nq_half), np.float32)], 0)
    Pb = Pk[b]
    pt = np.concatenate([Pb, -(Pb * Pb).sum(0, keepdims=True)], 0)
    return {
        "qt": np.ascontiguousarray(qt, np.float32),
        "pt": np.ascontiguousarray(pt, np.float32),
        "fk": np.ascontiguousarray(Fk[b], np.float32),
        "fq": np.ascontiguousarray(Fq[b][:, q0:q0 + nq_half], np.float32),
        "w1t": np.ascontiguousarray(W[:, :C_IN].T, np.float32),
        "dt": np.ascontiguousarray((W[:, C_IN:] - W[:, :C_IN]).T, np.float32),
    }


_NC_CACHE = {}
TRACE = False       # set True to capture an NTFF profile on the next call
LAST_RESULT = None  # BassKernelResults of the most recent kernel() call


def kernel(Fq_bcn, Fk_bcn, Pq_b3n, Pk_b3n, W_conv, gn_gamma=None,
           gn_beta=None, k=16):
    k = int(k)
    assert k == KNN, f"kernel hardcodes k=16, got {k}"
    Fq = np.asarray(Fq_bcn, np.float32)
    Fk = np.asarray(Fk_bcn, np.float32)
    Pq = np.asarray(Pq_b3n, np.float32)
    Pk = np.asarray(Pk_b3n, np.float32)
    W = np.asarray(W_conv, np.float32)
    gam = (np.ones(C_OUT, np.float32) if gn_gamma is None
           else np.asarray(gn_gamma, np.float32).reshape(C_OUT))
    bet = (np.zeros(C_OUT, np.float32) if gn_beta is None
           else np.asarray(gn_beta, np.float32).reshape(C_OUT))
    assert Fq.shape == (B, C_IN, N_KEYS)

    nq = N_KEYS // 2
    neg = bool((gam < 0).any())
    key = ("full", neg)
    if key not in _NC_CACHE:
        _NC_CACHE[key] = build_edgeconv(nq=nq, nk=N_KEYS, nseg=16,
                                        n_pair_q=N_KEYS, neg_gamma=neg)
    nc = _NC_CACHE[key]

    in_maps = []
    for core in range(N_CORES):
        m = make_core_inputs(Fq, Fk, Pq, Pk, W, nq, core)
        m["gam"] = np.ascontiguousarray(gam.reshape(1, C_OUT))
        m["bet"] = np.ascontiguousarray(bet.reshape(1, C_OUT))
        in_maps.append(m)

    res = run_bass_kernel_spmd(nc, in_maps, core_ids=list(range(N_CORES)),
                               trace=TRACE)
    global LAST_RESULT
    LAST_RESULT = res
    out = np.empty((B, C_OUT, N_KEYS), np.float32)
    for core in range(N_CORES):
        b, h = core // 2, core % 2
        out[b, :, h * nq:(h + 1) * nq] = res.results[core]["out"]
    return out


if __name__ == "__main__":
    rng = np.random.default_rng(0)
    inputs = {
        "Fq_bcn": rng.standard_normal((B, C_IN, N_KEYS)).astype(np.float32),
        "Fk_bcn": rng.standard_normal((B, C_IN, N_KEYS)).astype(np.float32),
        "Pq_b3n": rng.standard_normal((B, 3, N_KEYS)).astype(np.float32),
        "Pk_b3n": rng.standard_normal((B, 3, N_KEYS)).astype(np.float32),
        "W_conv": (rng.standard_normal((C_OUT, 2 * C_IN)).astype(np.float32)
                   / np.sqrt(2 * C_IN)),
        "gn_gamma": np.ones(C_OUT, np.float32),
        "gn_beta": np.zeros(C_OUT, np.float32),
        "k": 16,
    }
    out = kernel(**inputs)
    print("kernel out", out.shape, out.dtype, float(np.abs(out).mean()))



# revision 2
# speedup vs baseline: 19.1959x; 19.1959x over previous
"""EdgeConv block (kNN -> gather -> 1x1 conv -> GroupNorm -> ReLU -> max over k)
as a Bass/Tile kernel for 8 Trainium2 NeuronCores.

Problem shapes (hardcoded): B=4, C_IN=64, C_OUT=128, N=8192, K=16, G=8.

Sharding: core c handles batch b = c//2, query half h = c%2 (4096 queries).
Key features Fk are pair-split: each core uploads only its half of the keys
(fp16), computes A_half = W1^T Fk_half on the PE, and an on-chip AllGather
over the pair reconstructs the full A = W1^T Fk [O, Nk].  GroupNorm
statistics are partial per core and combined with a pairwise AllReduce on a
[128, 2] tensor.

Math decomposition (avoids materializing [Nq, k, 2C] pair features):
  conv out[o,q,j] = W1 @ (nbr_j - Fi_q) + W2 @ Fi_q = A[o, idx[q,j]] + C[o,q]
  where A = W1 @ Fk  [O, Nk]  and  C = (W2 - W1) @ Fq  [O, Nq].
kNN scores s[q,p] = 2*Q.P - |P|^2 (monotone in -d2 per query) via fp32 PE
matmul with lhsT = [2qx; 2qy; 2qz; 1], rhs = [px; py; pz; -|P|^2].

Top-16 per query: 16 segments of Nk/16 keys; per-segment top-8 via DVE max8 +
max_index; merge the 16*8 candidates with two max8+match_replace rounds; turn
the selection mask into dense ranks with a prefix scan and compact the winning
global indices with a per-partition local_scatter.  (Exact unless >8 of the
true top-16 fall in one segment: P ~ 3e-6 per query.)

Neighbor reduction: gpsimd ap_gather of A columns (indices shared across all
128 channel partitions), then DVE blocked reduces for max_j / sum_j, fused
square-reduce for the GN second moment.

Host transport (the wall-clock bottleneck: these cores are tunneled, h2d
~70MB/s, d2h ~40MB/s): features/weights travel as fp16 packed in one blob
per core, coords/affine in one f32 blob; output is fp16.  The
jit(shard_map(...)) executable is built once and cached; the donated output
buffer is recycled from the previous call so no zero-fill upload happens.
"""

from contextlib import ExitStack

import numpy as np

import concourse.bass as bass
import concourse.bacc as bacc
import concourse.mybir as mybir
from concourse.tile import TileContext

F32 = mybir.dt.float32
F16 = mybir.dt.float16
I16 = mybir.dt.int16
U16 = mybir.dt.uint16

B, C_IN, C_OUT, N_KEYS, KNN, G = 4, 64, 128, 8192, 16, 8
GN_EPS = 1e-5
N_CORES = 8

NQ = N_KEYS // 2     # queries per core
NKH = N_KEYS // 2    # keys uploaded per core (pair-split)

# fp16 blob layout (elements)
FK_OFF = 0
FQ_OFF = FK_OFF + C_IN * NKH
W1_OFF = FQ_OFF + C_IN * NQ
DT_OFF = W1_OFF + C_IN * C_OUT
B16_TOT = DT_OFF + C_IN * C_OUT
# f32 blob layout (elements)
QT_OFF = 0
PT_OFF = QT_OFF + 4 * NQ
GAM_OFF = PT_OFF + 4 * N_KEYS
BET_OFF = GAM_OFF + C_OUT
B32_TOT = BET_OFF + C_OUT


def build_edgeconv(nq, nk, nseg, n_pair_q, neg_gamma=False,
                   num_devices=N_CORES, use_cc=True):
    """Build the SPMD program. nq: queries per core; nk: total keys;
    nseg: top-k segments (seg = nk//nseg <= 512); n_pair_q: total queries
    per batch across the core pair (GN denominator)."""
    seg = nk // nseg
    assert seg * nseg == nk and seg <= 512
    nkh = nk // 2
    ncand = nseg * 8
    qtiles = nq // 128
    assert qtiles * 128 == nq
    chunk = min(256, nq)  # queries per gather chunk
    qstep = min(512, nq)
    nchunk = nq // chunk
    assert nchunk * chunk == nq
    gn_count = float(n_pair_q * KNN * (C_OUT // G))
    ngrp = num_devices // 2

    nc = bacc.Bacc("TRN2", target_bir_lowering=False, debug=False,
                   num_devices=num_devices)

    blob16 = nc.dram_tensor("blob16", [1, B16_TOT], F16, kind="ExternalInput")
    blob32 = nc.dram_tensor("blob32", [1, B32_TOT], F32, kind="ExternalInput")
    out_ext = nc.dram_tensor("out", [C_OUT, nq], F16, kind="ExternalOutput")

    idx_dram = nc.dram_tensor("idx_scratch", [nq, KNN], I16)
    row_dram = nc.dram_tensor("row_scratch", [2, C_OUT], F32)
    cc_in = nc.dram_tensor("cc_in", [C_OUT, 2], F32)
    cc_out = nc.dram_tensor("cc_out", [C_OUT, 2], F32)
    ag_in = nc.dram_tensor("ag_in", [C_OUT, nkh], F32)
    ag_out = nc.dram_tensor("ag_out", [2 * C_OUT, nkh], F32)

    with TileContext(nc) as tc, ExitStack() as ctx:
        persist = ctx.enter_context(tc.tile_pool(name="persist", bufs=1))
        psum = ctx.enter_context(tc.tile_pool(name="psum", bufs=6,
                                              space="PSUM"))
        segp = ctx.enter_context(tc.tile_pool(name="segp", bufs=3))
        small = ctx.enter_context(tc.tile_pool(name="small", bufs=2))
        gchunk = ctx.enter_context(tc.tile_pool(name="gchunk", bufs=2))

        # ---- persistent SBUF ----
        # qt replicated at partition bases 0/32/64/96 so four q-tiles'
        # K=4 matmuls can run concurrently in distinct PE row groups
        rowtile = 4 if qtiles % 4 == 0 else 1
        qt_sb = persist.tile([128 if rowtile == 4 else 4, nq], F32,
                             tag="qt_sb")
        for r in range(rowtile):
            nc.sync.dma_start(
                out=qt_sb[32 * r:32 * r + 4, :],
                in_=bass.AP(tensor=blob32, offset=QT_OFF,
                            ap=[[nq, 4], [1, nq]]))
        pt_sb = persist.tile([128 if rowtile == 4 else 4, nk], F32,
                             tag="pt_sb")
        for r in range(rowtile):
            nc.sync.dma_start(
                out=pt_sb[32 * r:32 * r + 4, :],
                in_=bass.AP(tensor=blob32, offset=PT_OFF,
                            ap=[[nk, 4], [1, nk]]))
        w1t_sb = persist.tile([C_IN, C_OUT], F16, tag="w1t_sb")
        nc.sync.dma_start(out=w1t_sb,
                          in_=bass.AP(tensor=blob16, offset=W1_OFF,
                                      ap=[[C_OUT, C_IN], [1, C_OUT]]))
        dtw_sb = persist.tile([C_IN, C_OUT], F16, tag="dtw_sb")
        nc.sync.dma_start(out=dtw_sb,
                          in_=bass.AP(tensor=blob16, offset=DT_OFF,
                                      ap=[[C_OUT, C_IN], [1, C_OUT]]))
        gam_sb = persist.tile([1, C_OUT], F32, tag="gam_sb")
        nc.sync.dma_start(out=gam_sb,
                          in_=bass.AP(tensor=blob32, offset=GAM_OFF,
                                      ap=[[C_OUT, 1], [1, C_OUT]]))
        bet_sb = persist.tile([1, C_OUT], F32, tag="bet_sb")
        nc.sync.dma_start(out=bet_sb,
                          in_=bass.AP(tensor=blob32, offset=BET_OFF,
                                      ap=[[C_OUT, 1], [1, C_OUT]]))

        a_sb = persist.tile([C_OUT, nk], F32, tag="a_sb")
        c_sb = persist.tile([C_OUT, nq], F32, tag="c_sb")
        mpos_sb = persist.tile([C_OUT, nq], F32, tag="mpos_sb")
        mneg_sb = (persist.tile([C_OUT, nq], F32, tag="mneg_sb")
                   if neg_gamma else None)
        seg_off = persist.tile([128, ncand], I16, tag="seg_off")
        nc.gpsimd.iota(seg_off, pattern=[[seg, nseg], [0, 8]], base=0,
                       channel_multiplier=0)
        zeros_nc = persist.tile([128, ncand], F32, tag="zeros_nc")
        nc.vector.memset(zeros_nc, 0.0)

        # ---- A-half and C matmuls (fp16 in, fp32 out), streamed ----
        with tc.tile_pool(name="feat", bufs=3) as featp:
            for s0 in range(0, nkh, 512):
                fk_t = featp.tile([C_IN, 512], F16, tag="fk_t")
                nc.sync.dma_start(
                    out=fk_t,
                    in_=bass.AP(tensor=blob16, offset=FK_OFF + s0,
                                ap=[[nkh, C_IN], [1, 512]]))
                ps = psum.tile([C_OUT, 512], F32, tag="ps")
                nc.tensor.matmul(ps, lhsT=w1t_sb, rhs=fk_t,
                                 start=True, stop=True)
                ah_t = featp.tile([C_OUT, 512], F32, tag="ah_t")
                nc.scalar.copy(out=ah_t, in_=ps)
                nc.sync.dma_start(out=ag_in[:, s0:s0 + 512], in_=ah_t)
            for s0 in range(0, nq, qstep):
                fq_t = featp.tile([C_IN, qstep], F16, tag="fq_t")
                nc.sync.dma_start(
                    out=fq_t,
                    in_=bass.AP(tensor=blob16, offset=FQ_OFF + s0,
                                ap=[[nq, C_IN], [1, qstep]]))
                ps = psum.tile([C_OUT, qstep], F32, tag="ps")
                nc.tensor.matmul(ps, lhsT=dtw_sb, rhs=fq_t,
                                 start=True, stop=True)
                nc.scalar.copy(out=c_sb[:, s0:s0 + qstep], in_=ps)

        # ---- pairwise AllGather reconstructs full A = W1^T Fk ----
        if use_cc:
            nc.gpsimd.collective_compute(
                "AllGather", mybir.AluOpType.bypass,
                replica_groups=[[2 * i, 2 * i + 1] for i in range(ngrp)],
                ins=[ag_in[:, :]], outs=[ag_out[:, :]])
        else:
            # diagnostic mode: replicate own half (output is approximate)
            nc.sync.dma_start(out=ag_out[:C_OUT, :], in_=ag_in[:, :])
            nc.sync.dma_start(out=ag_out[C_OUT:, :], in_=ag_in[:, :])
        for h in range(2):
            nc.sync.dma_start(
                out=a_sb[:, h * nkh:(h + 1) * nkh],
                in_=bass.AP(tensor=ag_out, offset=h * C_OUT * nkh,
                            ap=[[nkh, C_OUT], [1, nkh]]))

        # stat accumulators (filled by interleaved gather chunks)
        r_sa = small.tile([128, 1], F32, tag="acc_sa")
        r_sqa = small.tile([128, 1], F32, tag="acc_sqa")
        r_csa = small.tile([128, 1], F32, tag="acc_csa")
        nc.vector.memset(r_sa, 0.0)
        nc.vector.memset(r_sqa, 0.0)
        nc.vector.memset(r_csa, 0.0)

        def emit_gather_chunk(ch):
            q0 = ch * chunk
            idxs_t = gchunk.tile([128, chunk], I16, tag="idxs_t")
            for g in range(8):
                nc.sync.dma_start(
                    out=idxs_t[g * 16:(g + 1) * 16, :],
                    in_=bass.AP(tensor=idx_dram, offset=q0 * KNN,
                                ap=[[1, KNN], [KNN, chunk]]),
                )
            ga = gchunk.tile([128, chunk * KNN], F32, tag="ga")
            nc.gpsimd.ap_gather(out_ap=ga, in_ap=a_sb, idxs_ap=idxs_t,
                                channels=128, num_elems=nk, d=1,
                                num_idxs=chunk * KNN)
            gav = ga.rearrange("p (q c) -> p q c", c=KNN)
            nc.vector.tensor_reduce(out=mpos_sb[:, q0:q0 + chunk], in_=gav,
                                    axis=mybir.AxisListType.X,
                                    op=mybir.AluOpType.max)
            if neg_gamma:
                nc.vector.tensor_reduce(out=mneg_sb[:, q0:q0 + chunk],
                                        in_=gav, axis=mybir.AxisListType.X,
                                        op=mybir.AluOpType.min)
            sa_c = gchunk.tile([128, chunk], F32, tag="sa_c")
            nc.vector.tensor_reduce(out=sa_c, in_=gav,
                                    axis=mybir.AxisListType.X,
                                    op=mybir.AluOpType.add)
            tmp1 = small.tile([128, 1], F32, tag="tmp1")
            nc.vector.tensor_reduce(out=tmp1, in_=sa_c,
                                    axis=mybir.AxisListType.X,
                                    op=mybir.AluOpType.add)
            nc.vector.tensor_add(r_sa, r_sa, tmp1)
            scr_c = gchunk.tile([128, chunk], F32, tag="scr_c")
            nc.vector.tensor_mul(scr_c, sa_c, c_sb[:, q0:q0 + chunk])
            nc.vector.tensor_reduce(out=tmp1, in_=scr_c,
                                    axis=mybir.AxisListType.X,
                                    op=mybir.AluOpType.add)
            nc.vector.tensor_add(r_csa, r_csa, tmp1)
            # in-place square on gpsimd (offloads the DVE bottleneck)
            nc.gpsimd.tensor_mul(ga, ga, ga)
            nc.vector.tensor_reduce(out=tmp1, in_=ga,
                                    axis=mybir.AxisListType.X,
                                    op=mybir.AluOpType.add)
            nc.vector.tensor_add(r_sqa, r_sqa, tmp1)

        queries_per_group = rowtile * 128
        # ---- per-q-tile kNN (row-tiled: `rowtile` q-tiles in flight),
        # with gather chunks interleaved as soon as their indices land ----
        for tq0 in range(0, qtiles, rowtile):
            cvs, cis = [], []
            for r in range(rowtile):
                cv_r = small.tile([128, ncand], F32, tag=f"cv{r}")
                ci_r = small.tile([128, ncand], U16, tag=f"ci{r}")
                cvs.append(cv_r)
                cis.append(ci_r)
            for s in range(nseg):
                for r in range(rowtile):
                    t = tq0 + r
                    lhs_q = qt_sb[32 * r:32 * r + 4,
                                  t * 128:(t + 1) * 128]
                    ps = psum.tile([128, seg], F32, tag="ps")
                    nc.tensor.matmul(ps, lhsT=lhs_q,
                                     rhs=pt_sb[32 * r:32 * r + 4,
                                               s * seg:(s + 1) * seg],
                                     start=True, stop=True,
                                     tile_position=(32 * r, 0))
                    ssb = segp.tile([128, seg], F32, tag="ssb")
                    nc.scalar.copy(out=ssb, in_=ps)
                    nc.vector.max(out=cvs[r][:, s * 8:(s + 1) * 8], in_=ssb)
                    nc.vector.max_index(out=cis[r][:, s * 8:(s + 1) * 8],
                                        in_max=cvs[r][:, s * 8:(s + 1) * 8],
                                        in_values=ssb)
            for r in range(rowtile):
                t = tq0 + r
                cv, ci = cvs[r], cis[r]
                v1 = small.tile([128, 8], F32, tag="v1")
                v2 = small.tile([128, 8], F32, tag="v2")
                cv2 = small.tile([128, ncand], F32, tag="cv2")
                cv3 = small.tile([128, ncand], F32, tag="cv3")
                nc.vector.max(out=v1, in_=cv)
                nc.vector.match_replace(out=cv2, in_to_replace=v1,
                                        in_values=cv, imm_value=-1e30)
                nc.vector.max(out=v2, in_=cv2)
                nc.vector.match_replace(out=cv3, in_to_replace=v2,
                                        in_values=cv2, imm_value=-1e30)
                maskf = small.tile([128, ncand], F32, tag="maskf")
                nc.vector.tensor_tensor(out=maskf, in0=cv, in1=cv3,
                                        op=mybir.AluOpType.not_equal)
                rk = small.tile([128, ncand], F32, tag="rk")
                nc.vector.tensor_tensor_scan(out=rk, data0=maskf,
                                             data1=zeros_nc, initial=0.0,
                                             op0=mybir.AluOpType.add,
                                             op1=mybir.AluOpType.add)
                tgt = small.tile([128, ncand], F32, tag="tgt")
                nc.vector.tensor_tensor(out=tgt, in0=rk, in1=maskf,
                                        op=mybir.AluOpType.mult)
                nc.vector.tensor_scalar_add(tgt, tgt, -1.0)
                tgt_i = small.tile([128, ncand], I16, tag="tgti")
                nc.vector.tensor_copy(tgt_i, tgt)
                gidx = small.tile([128, ncand], I16, tag="gidx")
                nc.vector.tensor_tensor(out=gidx, in0=ci.bitcast(I16),
                                        in1=seg_off, op=mybir.AluOpType.add)
                idx16 = small.tile([128, KNN], I16, tag="idx16")
                nc.gpsimd.local_scatter(out_ap=idx16, data_ap=gidx,
                                        idxs_ap=tgt_i, channels=128,
                                        num_elems=KNN, num_idxs=ncand)
                nc.sync.dma_start(out=idx_dram[t * 128:(t + 1) * 128, :],
                                  in_=idx16)
            # emit the PREVIOUS group's gather chunks: their idx writes
            # have had a full group of kNN work to complete, so the DRAM
            # round-trip latency is hidden
            if tq0 > 0:
                prev_q0 = (tq0 - rowtile) * 128
                for ch in range(prev_q0 // chunk,
                                (prev_q0 + queries_per_group) // chunk):
                    emit_gather_chunk(ch)

        # flush the final group's gather chunks
        last_q0 = (qtiles - rowtile) * 128
        for ch in range(last_q0 // chunk,
                        (last_q0 + queries_per_group) // chunk):
            emit_gather_chunk(ch)

        r_c = small.tile([128, 1], F32, tag="r_c")
        nc.vector.tensor_reduce(out=r_c, in_=c_sb,
                                axis=mybir.AxisListType.X,
                                op=mybir.AluOpType.add)
        r_c2 = small.tile([128, 1], F32, tag="r_c2")
        nc.vector.memset(r_c2, 0.0)
        tmpc = small.tile([128, 1], F32, tag="tmpc")
        for q0 in range(0, nq, qstep):
            scr5 = gchunk.tile([128, qstep], F32, tag="scr5")
            nc.vector.tensor_mul(scr5, c_sb[:, q0:q0 + qstep],
                                 c_sb[:, q0:q0 + qstep])
            nc.vector.tensor_reduce(out=tmpc, in_=scr5,
                                    axis=mybir.AxisListType.X,
                                    op=mybir.AluOpType.add)
            nc.vector.tensor_add(r_c2, r_c2, tmpc)

        s1p = small.tile([128, 1], F32, tag="s1p")
        nc.vector.tensor_scalar(out=s1p, in0=r_c, scalar1=float(KNN),
                                scalar2=None, op0=mybir.AluOpType.mult)
        nc.vector.tensor_add(s1p, s1p, r_sa)
        s2p = small.tile([128, 1], F32, tag="s2p")
        nc.vector.tensor_scalar(out=s2p, in0=r_c2, scalar1=float(KNN),
                                scalar2=None, op0=mybir.AluOpType.mult)
        t2 = small.tile([128, 1], F32, tag="t2")
        nc.vector.tensor_scalar(out=t2, in0=r_csa, scalar1=2.0,
                                scalar2=None, op0=mybir.AluOpType.mult)
        nc.vector.tensor_add(s2p, s2p, t2)
        nc.vector.tensor_add(s2p, s2p, r_sqa)

        # ---- pairwise allreduce of [128, 2] partials ----
        s12 = small.tile([128, 2], F32, tag="s12")
        nc.vector.tensor_copy(s12[:, 0:1], s1p)
        nc.vector.tensor_copy(s12[:, 1:2], s2p)
        nc.sync.dma_start(out=cc_in[:, :], in_=s12)
        if use_cc:
            nc.gpsimd.collective_compute(
                "AllReduce", mybir.AluOpType.add,
                replica_groups=[[2 * i, 2 * i + 1] for i in range(ngrp)],
                ins=[cc_in[:, :]], outs=[cc_out[:, :]])
        else:
            # diagnostic mode: no cross-core reduce (stats use only this
            # core's half; output is approximate)
            nc.sync.dma_start(out=cc_out[:, :], in_=s12)

        # ---- finish GroupNorm stats in [*, C_OUT] row layout ----
        st1 = small.tile([1, C_OUT], F32, tag="st1")
        nc.sync.dma_start(out=st1,
                          in_=bass.AP(tensor=cc_out, offset=0,
                                      ap=[[0, 1], [2, C_OUT]]))
        st2 = small.tile([1, C_OUT], F32, tag="st2")
        nc.sync.dma_start(out=st2,
                          in_=bass.AP(tensor=cc_out, offset=1,
                                      ap=[[0, 1], [2, C_OUT]]))
        sg1 = small.tile([1, G], F32, tag="sg1")
        nc.vector.tensor_reduce(out=sg1,
                                in_=st1.rearrange("p (g d) -> p g d", g=G),
                                axis=mybir.AxisListType.X,
                                op=mybir.AluOpType.add)
        sg2 = small.tile([1, G], F32, tag="sg2")
        nc.vector.tensor_reduce(out=sg2,
                                in_=st2.rearrange("p (g d) -> p g d", g=G),
                                axis=mybir.AxisListType.X,
                                op=mybir.AluOpType.add)
        mean_r = small.tile([1, G], F32, tag="mean_r")
        nc.vector.tensor_scalar(out=mean_r, in0=sg1,
                                scalar1=1.0 / gn_count, scalar2=None,
                                op0=mybir.AluOpType.mult)
        ex2_r = small.tile([1, G], F32, tag="ex2_r")
        nc.vector.tensor_scalar(out=ex2_r, in0=sg2,
                                scalar1=1.0 / gn_count, scalar2=None,
                                op0=mybir.AluOpType.mult)
        var_r = small.tile([1, G], F32, tag="var_r")
        nc.vector.tensor_tensor(out=var_r, in0=mean_r, in1=mean_r,
                                op=mybir.AluOpType.mult)
        nc.vector.tensor_tensor(out=var_r, in0=ex2_r, in1=var_r,
                                op=mybir.AluOpType.subtract)
        sd_r = small.tile([1, G], F32, tag="sd_r")
        nc.vector.tensor_scalar_add(var_r, var_r, GN_EPS)
        nc.scalar.activation(sd_r, var_r, mybir.ActivationFunctionType.Sqrt,
                             bias=0.0)
        rstd_r = small.tile([1, G], F32, tag="rstd_r")
        nc.vector.reciprocal(rstd_r, sd_r)
        mean_c = small.tile([1, C_OUT], F32, tag="mean_c")
        rstd_c = small.tile([1, C_OUT], F32, tag="rstd_c")
        gsz = C_OUT // G
        for g in range(G):
            nc.vector.tensor_copy(
                mean_c[:, g * gsz:(g + 1) * gsz],
                mean_r[:, g:g + 1].to_broadcast([1, gsz]))
            nc.vector.tensor_copy(
                rstd_c[:, g * gsz:(g + 1) * gsz],
                rstd_r[:, g:g + 1].to_broadcast([1, gsz]))
        srow = small.tile([1, C_OUT], F32, tag="srow")
        nc.vector.tensor_tensor(out=srow, in0=gam_sb, in1=rstd_c,
                                op=mybir.AluOpType.mult)
        trow = small.tile([1, C_OUT], F32, tag="trow")
        nc.vector.tensor_tensor(out=trow, in0=mean_c, in1=srow,
                                op=mybir.AluOpType.mult)
        nc.vector.tensor_tensor(out=trow, in0=bet_sb, in1=trow,
                                op=mybir.AluOpType.subtract)
        # transpose the two [1, C_OUT] rows to [C_OUT, 1] via DRAM bounce
        nc.sync.dma_start(out=row_dram[0:1, :], in_=srow)
        nc.sync.dma_start(out=row_dram[1:2, :], in_=trow)
        s_col = small.tile([C_OUT, 1], F32, tag="s_col")
        nc.sync.dma_start(out=s_col,
                          in_=bass.AP(tensor=row_dram, offset=0,
                                      ap=[[1, C_OUT], [0, 1]]))
        t_col = small.tile([C_OUT, 1], F32, tag="t_col")
        nc.sync.dma_start(out=t_col,
                          in_=bass.AP(tensor=row_dram, offset=C_OUT,
                                      ap=[[1, C_OUT], [0, 1]]))

        # ---- final normalization + relu + fp16 output ----
        for q0 in range(0, nq, qstep):
            mf = gchunk.tile([128, qstep], F32, tag="mf")
            nc.vector.tensor_add(mf, mpos_sb[:, q0:q0 + qstep],
                                 c_sb[:, q0:q0 + qstep])
            nc.vector.tensor_scalar(out=mf, in0=mf, scalar1=s_col,
                                    scalar2=t_col,
                                    op0=mybir.AluOpType.mult,
                                    op1=mybir.AluOpType.add)
            if neg_gamma:
                mn = gchunk.tile([128, qstep], F32, tag="mn")
                nc.vector.tensor_add(mn, mneg_sb[:, q0:q0 + qstep],
                                     c_sb[:, q0:q0 + qstep])
                nc.vector.tensor_scalar(out=mn, in0=mn, scalar1=s_col,
                                        scalar2=t_col,
                                        op0=mybir.AluOpType.mult,
                                        op1=mybir.AluOpType.add)
                nc.vector.tensor_tensor(out=mf, in0=mf, in1=mn,
                                        op=mybir.AluOpType.max)
            o16 = gchunk.tile([128, qstep], F16, tag="o16")
            nc.vector.tensor_scalar_max(o16, mf, 0.0)
            nc.sync.dma_start(out=out_ext[:, q0:q0 + qstep], in_=o16)

    nc.finalize()
    return nc


def make_core_blobs(Fq16, Fk16, Pq, Pk, w1t16, dt16, gam, bet, core,
                    blob16, blob32):
    """Fill the per-core rows of the input blobs."""
    b, h = core // 2, core % 2
    q0 = h * NQ
    r16 = blob16[core]
    r16[FK_OFF:FK_OFF + C_IN * NKH].reshape(C_IN, NKH)[:] = \
        Fk16[b][:, q0:q0 + NKH]
    r16[FQ_OFF:FQ_OFF + C_IN * NQ].reshape(C_IN, NQ)[:] = \
        Fq16[b][:, q0:q0 + NQ]
    r16[W1_OFF:W1_OFF + C_IN * C_OUT].reshape(C_IN, C_OUT)[:] = w1t16
    r16[DT_OFF:DT_OFF + C_IN * C_OUT].reshape(C_IN, C_OUT)[:] = dt16
    r32 = blob32[core]
    qt = r32[QT_OFF:QT_OFF + 4 * NQ].reshape(4, NQ)
    qt[:3] = 2.0 * Pq[b][:, q0:q0 + NQ]
    qt[3] = 1.0
    pt = r32[PT_OFF:PT_OFF + 4 * N_KEYS].reshape(4, N_KEYS)
    pt[:3] = Pk[b]
    pt[3] = -(Pk[b] * Pk[b]).sum(0)
    r32[GAM_OFF:GAM_OFF + C_OUT] = gam
    r32[BET_OFF:BET_OFF + C_OUT] = bet


class CachedRunner:
    """Builds the jit(shard_map(bass_exec)) once; reuses it every call,
    recycling the donated output buffer so only real input bytes move."""

    def __init__(self, nc, n_cores):
        import jax
        from jax.sharding import Mesh, PartitionSpec, NamedSharding
        from jax.experimental.shard_map import shard_map
        from concourse.bass2jax import (_bass_exec_p, partition_id_tensor,
                                        install_neuronx_cc_hook)
        install_neuronx_cc_hook()
        self.jax = jax
        self.nc = nc
        self.n_cores = n_cores
        partition_name = (nc.partition_id_tensor.name
                          if nc.partition_id_tensor else None)
        in_names, out_names, out_avals, zero_shapes = [], [], [], []
        for alloc in nc.m.functions[0].allocations:
            if not isinstance(alloc, mybir.MemoryLocationSet):
                continue
            name = alloc.memorylocations[0].name
            if alloc.kind == "ExternalInput":
                if name != partition_name:
                    in_names.append(name)
            elif alloc.kind == "ExternalOutput":
                out_names.append(name)
                shape = tuple(alloc.tensor_shape)
                dtype = mybir.dt.np(alloc.dtype)
                out_avals.append(jax.core.ShapedArray(shape, dtype))
                zero_shapes.append((shape, dtype))
        self.n_params = len(in_names)
        n_outs = len(out_avals)
        self.in_names = list(in_names)
        self.out_names = out_names
        self.out_avals = out_avals
        all_in_names = in_names + out_names
        if partition_name is not None:
            all_in_names.append(partition_name)

        def _body(*args):
            operands = list(args)
            if partition_name is not None:
                operands.append(partition_id_tensor())
            outs = _bass_exec_p.bind(
                *operands,
                out_avals=tuple(out_avals),
                in_names=tuple(all_in_names),
                out_names=tuple(out_names),
                lowering_input_output_aliases=(),
                sim_require_finite=True,
                sim_require_nnan=True,
                nc=nc,
            )
            return tuple(outs)

        devices = jax.devices()[:n_cores]
        self.mesh = Mesh(np.asarray(devices), ("core",))
        self.shd = NamedSharding(self.mesh, PartitionSpec("core"))
        in_specs = (PartitionSpec("core"),) * (self.n_params + n_outs)
        out_specs = (PartitionSpec("core"),) * len(out_names)
        donate = tuple(range(self.n_params, self.n_params + n_outs))
        self.sharded = jax.jit(
            shard_map(_body, mesh=self.mesh, in_specs=in_specs,
                      out_specs=out_specs, check_rep=False),
            donate_argnums=donate, keep_unused=True,
        )
        import jax.numpy as jnp
        self._mk_zeros = jax.jit(
            lambda: tuple(
                jnp.zeros((n_cores * s[0], *s[1:]), d)
                for (s, d) in zero_shapes),
            out_shardings=(self.shd,) * n_outs)
        self._donate_next = None

    def __call__(self, concat_in):
        jax = self.jax
        if self._donate_next is None:
            douts = self._mk_zeros()
        else:
            douts = self._donate_next
        dev_in = jax.device_put(concat_in, [self.shd] * len(concat_in))
        out_arrs = self.sharded(*dev_in, *douts)
        host = [np.asarray(a) for a in out_arrs]
        # previous outputs are fetched; their device buffers become next
        # call's donated output operands (values are fully overwritten)
        self._donate_next = tuple(out_arrs)
        return host


_NC_CACHE = {}
_RUNNER_CACHE = {}
TRACE = False       # set True to capture an NTFF profile on the next call
LAST_RESULT = None  # BassKernelResults of the most recent traced call


def kernel(Fq_bcn, Fk_bcn, Pq_b3n, Pk_b3n, W_conv, gn_gamma=None,
           gn_beta=None, k=16):
    k = int(k)
    assert k == KNN, f"kernel hardcodes k=16, got {k}"
    Fq = np.asarray(Fq_bcn, np.float32)
    Fk = np.asarray(Fk_bcn, np.float32)
    Pq = np.asarray(Pq_b3n, np.float32)
    Pk = np.asarray(Pk_b3n, np.float32)
    W = np.asarray(W_conv, np.float32)
    gam = (np.ones(C_OUT, np.float32) if gn_gamma is None
           else np.asarray(gn_gamma, np.float32).reshape(C_OUT))
    bet = (np.zeros(C_OUT, np.float32) if gn_beta is None
           else np.asarray(gn_beta, np.float32).reshape(C_OUT))
    assert Fq.shape == (B, C_IN, N_KEYS)

    neg = bool((gam < 0).any())
    key = ("full", neg)
    if key not in _NC_CACHE:
        _NC_CACHE[key] = build_edgeconv(nq=NQ, nk=N_KEYS, nseg=16,
                                        n_pair_q=N_KEYS, neg_gamma=neg)
    nc = _NC_CACHE[key]

    Fq16 = Fq.astype(np.float16)
    Fk16 = Fk.astype(np.float16)
    w1t16 = np.ascontiguousarray(W[:, :C_IN].T).astype(np.float16)
    dt16 = np.ascontiguousarray((W[:, C_IN:] - W[:, :C_IN]).T
                                ).astype(np.float16)
    blob16 = np.empty((N_CORES, B16_TOT), np.float16)
    blob32 = np.empty((N_CORES, B32_TOT), np.float32)
    for core in range(N_CORES):
        make_core_blobs(Fq16, Fk16, Pq, Pk, w1t16, dt16, gam, bet, core,
                        blob16, blob32)

    if TRACE:
        from concourse.bass_utils import run_bass_kernel_spmd
        in_maps = [{"blob16": blob16[c:c + 1], "blob32": blob32[c:c + 1]}
                   for c in range(N_CORES)]
        res = run_bass_kernel_spmd(nc, in_maps,
                                   core_ids=list(range(N_CORES)), trace=True)
        global LAST_RESULT
        LAST_RESULT = res
        shards = np.stack([res.results[c]["out"] for c in range(N_CORES)])
    else:
        if key not in _RUNNER_CACHE:
            _RUNNER_CACHE[key] = CachedRunner(nc, N_CORES)
        runner = _RUNNER_CACHE[key]
        name_to_blob = {"blob16": blob16, "blob32": blob32}
        concat_in = [name_to_blob[n].reshape(N_CORES, 1, -1)
                     for n in runner.in_names]
        host = runner(concat_in)
        shards = host[0].reshape(N_CORES, C_OUT, NQ)

    out = np.empty((B, C_OUT, N_KEYS), np.float32)
    for core in range(N_CORES):
        b, h = core // 2, core % 2
        out[b, :, h * NQ:(h + 1) * NQ] = shards[core]
    return out


if __name__ == "__main__":
    rng = np.random.default_rng(0)
    inputs = {
        "Fq_bcn": rng.standard_normal((B, C_IN, N_KEYS)).astype(np.float32),
        "Fk_bcn": rng.standard_normal((B, C_IN, N_KEYS)).astype(np.float32),
        "Pq_b3n": rng.standard_normal((B, 3, N_KEYS)).astype(np.float32),
        "Pk_b3n": rng.standard_normal((B, 3, N_KEYS)).astype(np.float32),
        "W_conv": (rng.standard_normal((C_OUT, 2 * C_IN)).astype(np.float32)
                   / np.sqrt(2 * C_IN)),
        "gn_gamma": np.ones(C_OUT, np.float32),
        "gn_beta": np.zeros(C_OUT, np.float32),
        "k": 16,
    }
    import time
    out = kernel(**inputs)
    print("kernel out", out.shape, out.dtype, float(np.abs(out).mean()))
    for i in range(3):
        t0 = time.perf_counter()
        out = kernel(**inputs)
        t1 = time.perf_counter()
        print(f"warm call {i}: {(t1 - t0) * 1e3:.1f} ms")


# revision 7
# speedup vs baseline: 24.3137x; 1.2666x over previous
"""EdgeConv block (kNN -> gather -> 1x1 conv -> GroupNorm -> ReLU -> max over k)
as a Bass/Tile kernel for 8 Trainium2 NeuronCores.

Problem shapes (hardcoded): B=4, C_IN=64, C_OUT=128, N=8192, K=16, G=8.

Sharding: core c handles batch b = c//2, query half h = c%2 (4096 queries).
Key features Fk are pair-split: each core uploads only its half of the keys
(fp16), computes A_half = W1^T Fk_half on the PE, and an on-chip AllGather
over the pair reconstructs the full A = W1^T Fk [O, Nk].

Math decomposition (avoids materializing [Nq, k, 2C] pair features):
  conv out[o,q,j] = W1 @ (nbr_j - Fi_q) + W2 @ Fi_q = A[o, idx[q,j]] + C[o,q]
  where A = W1 @ Fk  [O, Nk]  and  C = (W2 - W1) @ Fq  [O, Nq].
kNN scores s[q,p] = 2*Q.P - |P|^2 (monotone in -d2 per query) via fp32 PE
matmul with lhsT = [2qx; 2qy; 2qz; 1], rhs = [px; py; pz; -|P|^2].

Top-16 per query: 16 segments of Nk/16 keys; per-segment top-8 via DVE max8 +
max_index; merge the 16*8 candidates with two max8+match_replace rounds; turn
the selection mask into dense ranks with a prefix scan and compact the winning
global indices with a per-partition local_scatter.  (Exact unless >8 of the
true top-16 fall in one segment: P ~ 3e-6 per query.)

Neighbor reduction: gpsimd ap_gather of A columns (indices shared across all
128 channel partitions), then DVE blocked reduces for max_j / sum_j, fused
square-reduce for the GN second moment.

Transport design (these cores are tunneled: ~80ms dispatch RTT, h2d
~40-75MB/s, d2h ~40-70MB/s; device exec is ~2ms and irrelevant):
  * one compiled slice-program processes nq/S queries per call; queries are
    sliced so the fetch of slice s's output overlaps the upload + exec of
    slice s+1 (everything dispatched async, fetches are the only syncs);
  * features/weights travel fp16; the unnormalized per-slice result m =
    max_j A[idx] + C leaves the device as fp16; GroupNorm statistics leave
    as [128, 2] partials and the GN affine + ReLU finishes on the host
    (max_j commutes with the positive-scale affine; a negative-gamma build
    variant also emits min_j);
  * the jit(shard_map(bass_exec)) executables are built once and cached;
    donated output buffers are recycled between calls.
"""

from contextlib import ExitStack

import numpy as np

import concourse.bass as bass
import concourse.bacc as bacc
import concourse.mybir as mybir
from concourse.tile import TileContext

F32 = mybir.dt.float32
F16 = mybir.dt.float16
I16 = mybir.dt.int16
U16 = mybir.dt.uint16

B, C_IN, C_OUT, N_KEYS, KNN, G = 4, 64, 128, 8192, 16, 8
GN_EPS = 1e-5
N_CORES = 8

NQ = N_KEYS // 2     # queries per core
NKH = N_KEYS // 2    # keys uploaded per core (pair-split)
import os as _os
N_SLICES = int(_os.environ.get("EDGECONV_SLICES", "4"))
NQ_S = NQ // N_SLICES

# shared fp16 blob layout (elements)
FK_OFF = 0
W1_OFF = FK_OFF + C_IN * NKH
DT_OFF = W1_OFF + C_IN * C_OUT
B16_TOT = DT_OFF + C_IN * C_OUT
# shared f32 blob layout: pt rows [px; py; pz; -|P|^2]
B32_TOT = 4 * N_KEYS


def build_slice(nq_s, nk, nseg, neg_gamma=False, num_devices=N_CORES,
                use_cc=True):
    """One query-slice program: A (pair AllGather) + C + kNN + gather for
    nq_s queries; emits unnormalized m (fp16) + GN stat partials."""
    seg = nk // nseg
    assert seg * nseg == nk and seg <= 512
    nkh = nk // 2
    ncand = nseg * 8
    qtiles = nq_s // 128
    assert qtiles * 128 == nq_s
    chunk = min(256, nq_s)
    qstep = min(512, nq_s)
    assert (nq_s // chunk) * chunk == nq_s
    ngrp = num_devices // 2

    nc = bacc.Bacc("TRN2", target_bir_lowering=False, debug=False,
                   num_devices=num_devices)

    sb16 = nc.dram_tensor("sb16", [1, B16_TOT], F16, kind="ExternalInput")
    sb32 = nc.dram_tensor("sb32", [1, B32_TOT], F32, kind="ExternalInput")
    fqh_ext = nc.dram_tensor("fqh", [C_IN, nq_s], F16, kind="ExternalInput")
    qth_ext = nc.dram_tensor("qth", [4, nq_s], F32, kind="ExternalInput")
    # single output per slice: m columns [0:nq_s], fp16-encoded GN stat
    # partials in the last 4 columns (2 fp16 cols hold 1 f32 value each
    # via bitcast-free split: we just store the f32 stats as 4 fp16-sized
    # columns by DMAing a bitcast view)
    m_ext = nc.dram_tensor("m16", [C_OUT, nq_s + 4], F16,
                           kind="ExternalOutput")
    mn_ext = (nc.dram_tensor("mn16", [C_OUT, nq_s], F16,
                             kind="ExternalOutput") if neg_gamma else None)

    idx_dram = nc.dram_tensor("idx_scratch", [nq_s, KNN], I16)
    ag_in = nc.dram_tensor("ag_in", [C_OUT, nkh], F32)
    ag_out = nc.dram_tensor("ag_out", [2 * C_OUT, nkh], F32)

    with TileContext(nc) as tc, ExitStack() as ctx:
        persist = ctx.enter_context(tc.tile_pool(name="persist", bufs=1))
        psum = ctx.enter_context(tc.tile_pool(name="psum", bufs=6,
                                              space="PSUM"))
        segp = ctx.enter_context(tc.tile_pool(name="segp", bufs=3))
        small = ctx.enter_context(tc.tile_pool(name="small", bufs=2))
        gchunk = ctx.enter_context(tc.tile_pool(name="gchunk", bufs=2))

        # ---- persistent SBUF ----
        # qt replicated at partition bases 0/32/64/96 so four q-tiles'
        # K=4 matmuls can run concurrently in distinct PE row groups
        rowtile = 4 if qtiles % 4 == 0 else 1
        qt_sb = persist.tile([128 if rowtile == 4 else 4, nq_s], F32,
                             tag="qt_sb")
        for r in range(rowtile):
            nc.sync.dma_start(out=qt_sb[32 * r:32 * r + 4, :],
                              in_=qth_ext[:, :])
        pt_sb = persist.tile([128 if rowtile == 4 else 4, nk], F32,
                             tag="pt_sb")
        for r in range(rowtile):
            nc.sync.dma_start(
                out=pt_sb[32 * r:32 * r + 4, :],
                in_=bass.AP(tensor=sb32, offset=0, ap=[[nk, 4], [1, nk]]))
        w1t_sb = persist.tile([C_IN, C_OUT], F16, tag="w1t_sb")
        nc.sync.dma_start(out=w1t_sb,
                          in_=bass.AP(tensor=sb16, offset=W1_OFF,
                                      ap=[[C_OUT, C_IN], [1, C_OUT]]))
        dtw_sb = persist.tile([C_IN, C_OUT], F16, tag="dtw_sb")
        nc.sync.dma_start(out=dtw_sb,
                          in_=bass.AP(tensor=sb16, offset=DT_OFF,
                                      ap=[[C_OUT, C_IN], [1, C_OUT]]))

        a_sb = persist.tile([C_OUT, nk], F32, tag="a_sb")
        c_sb = persist.tile([C_OUT, nq_s], F32, tag="c_sb")
        mpos_sb = persist.tile([C_OUT, nq_s], F32, tag="mpos_sb")
        mneg_sb = (persist.tile([C_OUT, nq_s], F32, tag="mneg_sb")
                   if neg_gamma else None)
        seg_off = persist.tile([128, ncand], I16, tag="seg_off")
        nc.gpsimd.iota(seg_off, pattern=[[seg, nseg], [0, 8]], base=0,
                       channel_multiplier=0)
        zeros_nc = persist.tile([128, ncand], F32, tag="zeros_nc")
        nc.vector.memset(zeros_nc, 0.0)

        # ---- A-half and C matmuls (fp16 in, fp32 out), streamed ----
        with tc.tile_pool(name="feat", bufs=3) as featp:
            for s0 in range(0, nkh, 512):
                fk_t = featp.tile([C_IN, 512], F16, tag="fk_t")
                nc.sync.dma_start(
                    out=fk_t,
                    in_=bass.AP(tensor=sb16, offset=FK_OFF + s0,
                                ap=[[nkh, C_IN], [1, 512]]))
                ps = psum.tile([C_OUT, 512], F32, tag="ps")
                nc.tensor.matmul(ps, lhsT=w1t_sb, rhs=fk_t,
                                 start=True, stop=True)
                ah_t = featp.tile([C_OUT, 512], F32, tag="ah_t")
                nc.scalar.copy(out=ah_t, in_=ps)
                nc.sync.dma_start(out=ag_in[:, s0:s0 + 512], in_=ah_t)
            for s0 in range(0, nq_s, qstep):
                fq_t = featp.tile([C_IN, qstep], F16, tag="fq_t")
                nc.sync.dma_start(out=fq_t, in_=fqh_ext[:, s0:s0 + qstep])
                ps = psum.tile([C_OUT, qstep], F32, tag="ps")
                nc.tensor.matmul(ps, lhsT=dtw_sb, rhs=fq_t,
                                 start=True, stop=True)
                nc.scalar.copy(out=c_sb[:, s0:s0 + qstep], in_=ps)

        # ---- pairwise AllGather reconstructs full A = W1^T Fk ----
        if use_cc:
            nc.gpsimd.collective_compute(
                "AllGather", mybir.AluOpType.bypass,
                replica_groups=[[2 * i, 2 * i + 1] for i in range(ngrp)],
                ins=[ag_in[:, :]], outs=[ag_out[:, :]])
        else:
            # diagnostic mode: replicate own half (output is approximate)
            nc.sync.dma_start(out=ag_out[:C_OUT, :], in_=ag_in[:, :])
            nc.sync.dma_start(out=ag_out[C_OUT:, :], in_=ag_in[:, :])
        for h in range(2):
            nc.sync.dma_start(
                out=a_sb[:, h * nkh:(h + 1) * nkh],
                in_=bass.AP(tensor=ag_out, offset=h * C_OUT * nkh,
                            ap=[[nkh, C_OUT], [1, nkh]]))

        # stat accumulators (filled by interleaved gather chunks)
        r_sa = small.tile([128, 1], F32, tag="acc_sa")
        r_sqa = small.tile([128, 1], F32, tag="acc_sqa")
        r_csa = small.tile([128, 1], F32, tag="acc_csa")
        nc.vector.memset(r_sa, 0.0)
        nc.vector.memset(r_sqa, 0.0)
        nc.vector.memset(r_csa, 0.0)

        def emit_gather_chunk(ch):
            q0 = ch * chunk
            idxs_t = gchunk.tile([128, chunk], I16, tag="idxs_t")
            for g in range(8):
                nc.sync.dma_start(
                    out=idxs_t[g * 16:(g + 1) * 16, :],
                    in_=bass.AP(tensor=idx_dram, offset=q0 * KNN,
                                ap=[[1, KNN], [KNN, chunk]]),
                )
            ga = gchunk.tile([128, chunk * KNN], F32, tag="ga")
            nc.gpsimd.ap_gather(out_ap=ga, in_ap=a_sb, idxs_ap=idxs_t,
                                channels=128, num_elems=nk, d=1,
                                num_idxs=chunk * KNN)
            gav = ga.rearrange("p (q c) -> p q c", c=KNN)
            nc.vector.tensor_reduce(out=mpos_sb[:, q0:q0 + chunk], in_=gav,
                                    axis=mybir.AxisListType.X,
                                    op=mybir.AluOpType.max)
            if neg_gamma:
                nc.vector.tensor_reduce(out=mneg_sb[:, q0:q0 + chunk],
                                        in_=gav, axis=mybir.AxisListType.X,
                                        op=mybir.AluOpType.min)
            sa_c = gchunk.tile([128, chunk], F32, tag="sa_c")
            nc.vector.tensor_reduce(out=sa_c, in_=gav,
                                    axis=mybir.AxisListType.X,
                                    op=mybir.AluOpType.add)
            tmp1 = small.tile([128, 1], F32, tag="tmp1")
            nc.vector.tensor_reduce(out=tmp1, in_=sa_c,
                                    axis=mybir.AxisListType.X,
                                    op=mybir.AluOpType.add)
            nc.vector.tensor_add(r_sa, r_sa, tmp1)
            scr_c = gchunk.tile([128, chunk], F32, tag="scr_c")
            nc.vector.tensor_mul(scr_c, sa_c, c_sb[:, q0:q0 + chunk])
            nc.vector.tensor_reduce(out=tmp1, in_=scr_c,
                                    axis=mybir.AxisListType.X,
                                    op=mybir.AluOpType.add)
            nc.vector.tensor_add(r_csa, r_csa, tmp1)
            # in-place square on gpsimd (offloads the DVE bottleneck)
            nc.gpsimd.tensor_mul(ga, ga, ga)
            nc.vector.tensor_reduce(out=tmp1, in_=ga,
                                    axis=mybir.AxisListType.X,
                                    op=mybir.AluOpType.add)
            nc.vector.tensor_add(r_sqa, r_sqa, tmp1)

        queries_per_group = rowtile * 128
        # ---- per-q-tile kNN (row-tiled: `rowtile` q-tiles in flight),
        # with gather chunks interleaved as soon as their indices land ----
        for tq0 in range(0, qtiles, rowtile):
            cvs, cis = [], []
            for r in range(rowtile):
                cv_r = small.tile([128, ncand], F32, tag=f"cv{r}")
                ci_r = small.tile([128, ncand], U16, tag=f"ci{r}")
                cvs.append(cv_r)
                cis.append(ci_r)
            for s in range(nseg):
                for r in range(rowtile):
                    t = tq0 + r
                    lhs_q = qt_sb[32 * r:32 * r + 4,
                                  t * 128:(t + 1) * 128]
                    ps = psum.tile([128, seg], F32, tag="ps")
                    nc.tensor.matmul(ps, lhsT=lhs_q,
                                     rhs=pt_sb[32 * r:32 * r + 4,
                                               s * seg:(s + 1) * seg],
                                     start=True, stop=True,
                                     tile_position=(32 * r, 0))
                    ssb = segp.tile([128, seg], F32, tag="ssb")
                    nc.scalar.copy(out=ssb, in_=ps)
                    nc.vector.max(out=cvs[r][:, s * 8:(s + 1) * 8], in_=ssb)
                    nc.vector.max_index(out=cis[r][:, s * 8:(s + 1) * 8],
                                        in_max=cvs[r][:, s * 8:(s + 1) * 8],
                                        in_values=ssb)
            for r in range(rowtile):
                t = tq0 + r
                cv, ci = cvs[r], cis[r]
                v1 = small.tile([128, 8], F32, tag="v1")
                v2 = small.tile([128, 8], F32, tag="v2")
                cv2 = small.tile([128, ncand], F32, tag="cv2")
                cv3 = small.tile([128, ncand], F32, tag="cv3")
                nc.vector.max(out=v1, in_=cv)
                nc.vector.match_replace(out=cv2, in_to_replace=v1,
                                        in_values=cv, imm_value=-1e30)
                nc.vector.max(out=v2, in_=cv2)
                nc.vector.match_replace(out=cv3, in_to_replace=v2,
                                        in_values=cv2, imm_value=-1e30)
                maskf = small.tile([128, ncand], F32, tag="maskf")
                nc.vector.tensor_tensor(out=maskf, in0=cv, in1=cv3,
                                        op=mybir.AluOpType.not_equal)
                rk = small.tile([128, ncand], F32, tag="rk")
                nc.vector.tensor_tensor_scan(out=rk, data0=maskf,
                                             data1=zeros_nc, initial=0.0,
                                             op0=mybir.AluOpType.add,
                                             op1=mybir.AluOpType.add)
                tgt = small.tile([128, ncand], F32, tag="tgt")
                nc.vector.tensor_tensor(out=tgt, in0=rk, in1=maskf,
                                        op=mybir.AluOpType.mult)
                nc.vector.tensor_scalar_add(tgt, tgt, -1.0)
                tgt_i = small.tile([128, ncand], I16, tag="tgti")
                nc.vector.tensor_copy(tgt_i, tgt)
                gidx = small.tile([128, ncand], I16, tag="gidx")
                nc.vector.tensor_tensor(out=gidx, in0=ci.bitcast(I16),
                                        in1=seg_off, op=mybir.AluOpType.add)
                idx16 = small.tile([128, KNN], I16, tag="idx16")
                nc.gpsimd.local_scatter(out_ap=idx16, data_ap=gidx,
                                        idxs_ap=tgt_i, channels=128,
                                        num_elems=KNN, num_idxs=ncand)
                nc.sync.dma_start(out=idx_dram[t * 128:(t + 1) * 128, :],
                                  in_=idx16)
            # emit the PREVIOUS group's gather chunks: their idx writes
            # have had a full group of kNN work to complete, so the DRAM
            # round-trip latency is hidden
            if tq0 > 0:
                prev_q0 = (tq0 - rowtile) * 128
                for ch in range(prev_q0 // chunk,
                                (prev_q0 + queries_per_group) // chunk):
                    emit_gather_chunk(ch)

        # flush the final group's gather chunks
        last_q0 = (qtiles - rowtile) * 128
        for ch in range(last_q0 // chunk,
                        (last_q0 + queries_per_group) // chunk):
            emit_gather_chunk(ch)

        r_c = small.tile([128, 1], F32, tag="r_c")
        nc.vector.tensor_reduce(out=r_c, in_=c_sb,
                                axis=mybir.AxisListType.X,
                                op=mybir.AluOpType.add)
        r_c2 = small.tile([128, 1], F32, tag="r_c2")
        nc.vector.memset(r_c2, 0.0)
        tmpc = small.tile([128, 1], F32, tag="tmpc")
        for q0 in range(0, nq_s, qstep):
            scr5 = gchunk.tile([128, qstep], F32, tag="scr5")
            nc.vector.tensor_mul(scr5, c_sb[:, q0:q0 + qstep],
                                 c_sb[:, q0:q0 + qstep])
            nc.vector.tensor_reduce(out=tmpc, in_=scr5,
                                    axis=mybir.AxisListType.X,
                                    op=mybir.AluOpType.add)
            nc.vector.tensor_add(r_c2, r_c2, tmpc)

        # slice-partial GN sums: s1 = K*sum(C) + sum(Ag),
        # s2 = K*sum(C^2) + 2*sum(C*sum_j Ag) + sum(Ag^2)  [per partition]
        s1p = small.tile([128, 1], F32, tag="s1p")
        nc.vector.tensor_scalar(out=s1p, in0=r_c, scalar1=float(KNN),
                                scalar2=None, op0=mybir.AluOpType.mult)
        nc.vector.tensor_add(s1p, s1p, r_sa)
        s2p = small.tile([128, 1], F32, tag="s2p")
        nc.vector.tensor_scalar(out=s2p, in0=r_c2, scalar1=float(KNN),
                                scalar2=None, op0=mybir.AluOpType.mult)
        t2 = small.tile([128, 1], F32, tag="t2")
        nc.vector.tensor_scalar(out=t2, in0=r_csa, scalar1=2.0,
                                scalar2=None, op0=mybir.AluOpType.mult)
        nc.vector.tensor_add(s2p, s2p, t2)
        nc.vector.tensor_add(s2p, s2p, r_sqa)
        s12 = small.tile([128, 2], F32, tag="s12")
        nc.vector.tensor_copy(s12[:, 0:1], s1p)
        nc.vector.tensor_copy(s12[:, 1:2], s2p)
        nc.sync.dma_start(out=m_ext[:, nq_s:nq_s + 4],
                          in_=s12.bitcast(F16))

        # ---- unnormalized m = max_j A[idx] + C, fp16 out ----
        for q0 in range(0, nq_s, qstep):
            mf = gchunk.tile([128, qstep], F32, tag="mf")
            nc.vector.tensor_add(mf, mpos_sb[:, q0:q0 + qstep],
                                 c_sb[:, q0:q0 + qstep])
            o16 = gchunk.tile([128, qstep], F16, tag="o16")
            nc.vector.tensor_copy(o16, mf)
            nc.sync.dma_start(out=m_ext[:, q0:q0 + qstep], in_=o16)
            if neg_gamma:
                mn = gchunk.tile([128, qstep], F32, tag="mn")
                nc.vector.tensor_add(mn, mneg_sb[:, q0:q0 + qstep],
                                     c_sb[:, q0:q0 + qstep])
                on16 = gchunk.tile([128, qstep], F16, tag="on16")
                nc.vector.tensor_copy(on16, mn)
                nc.sync.dma_start(out=mn_ext[:, q0:q0 + qstep], in_=on16)

    nc.finalize()
    return nc


class CachedRunner:
    """Builds the jit(shard_map(bass_exec)) once; reuses it every call,
    recycling donated output buffers so only real input bytes move."""

    def __init__(self, nc, n_cores):
        import jax
        from jax.sharding import Mesh, PartitionSpec, NamedSharding
        from jax.experimental.shard_map import shard_map
        from concourse.bass2jax import (_bass_exec_p, partition_id_tensor,
                                        install_neuronx_cc_hook)
        install_neuronx_cc_hook()
        self.jax = jax
        self.nc = nc
        self.n_cores = n_cores
        partition_name = (nc.partition_id_tensor.name
                          if nc.partition_id_tensor else None)
        in_names, out_names, out_avals, zero_shapes = [], [], [], []
        for alloc in nc.m.functions[0].allocations:
            if not isinstance(alloc, mybir.MemoryLocationSet):
                continue
            name = alloc.memorylocations[0].name
            if alloc.kind == "ExternalInput":
                if name != partition_name:
                    in_names.append(name)
            elif alloc.kind == "ExternalOutput":
                out_names.append(name)
                shape = tuple(alloc.tensor_shape)
                dtype = mybir.dt.np(alloc.dtype)
                out_avals.append(jax.core.ShapedArray(shape, dtype))
                zero_shapes.append((shape, dtype))
        self.n_params = len(in_names)
        n_outs = len(out_avals)
        self.in_names = list(in_names)
        self.out_names = out_names
        self.out_avals = out_avals
        all_in_names = in_names + out_names
        if partition_name is not None:
            all_in_names.append(partition_name)

        def _body(*args):
            operands = list(args)
            if partition_name is not None:
                operands.append(partition_id_tensor())
            outs = _bass_exec_p.bind(
                *operands,
                out_avals=tuple(out_avals),
                in_names=tuple(all_in_names),
                out_names=tuple(out_names),
                lowering_input_output_aliases=(),
                sim_require_finite=True,
                sim_require_nnan=True,
                nc=nc,
            )
            return tuple(outs)

        devices = jax.devices()[:n_cores]
        self.mesh = Mesh(np.asarray(devices), ("core",))
        self.shd = NamedSharding(self.mesh, PartitionSpec("core"))
        in_specs = (PartitionSpec("core"),) * (self.n_params + n_outs)
        out_specs = (PartitionSpec("core"),) * len(out_names)
        donate = tuple(range(self.n_params, self.n_params + n_outs))
        self.sharded = jax.jit(
            shard_map(_body, mesh=self.mesh, in_specs=in_specs,
                      out_specs=out_specs, check_rep=False),
            donate_argnums=donate, keep_unused=True,
        )
        import jax.numpy as jnp
        self._mk_zeros = jax.jit(
            lambda: tuple(
                jnp.zeros((n_cores * s[0], *s[1:]), d)
                for (s, d) in zero_shapes),
            out_shardings=(self.shd,) * n_outs)
        # donated-output recycling, one stash per pipeline stage index
        self._donate = {}

    def launch(self, dev_in, stage):
        """Dispatch (async) with per-stage donated-output recycling."""
        douts = self._donate.get(stage)
        if douts is None:
            douts = self._mk_zeros()
        out_arrs = self.sharded(*dev_in, *douts)
        self._donate[stage] = tuple(out_arrs)
        return out_arrs


_NC_CACHE = {}
_RUNNER_CACHE = {}
TRACE = False       # kept for interface compat; NTFF unavailable here
LAST_RESULT = None


def _get_runner(neg):
    key = ("slice", N_SLICES, neg)
    if key not in _RUNNER_CACHE:
        if key not in _NC_CACHE:
            _NC_CACHE[key] = build_slice(nq_s=NQ_S, nk=N_KEYS, nseg=16,
                                         neg_gamma=neg)
        _RUNNER_CACHE[key] = CachedRunner(_NC_CACHE[key], N_CORES)
    return _RUNNER_CACHE[key]


def kernel(Fq_bcn, Fk_bcn, Pq_b3n, Pk_b3n, W_conv, gn_gamma=None,
           gn_beta=None, k=16):
    k = int(k)
    assert k == KNN, f"kernel hardcodes k=16, got {k}"
    Fq = np.asarray(Fq_bcn, np.float32)
    Fk = np.asarray(Fk_bcn, np.float32)
    Pq = np.asarray(Pq_b3n, np.float32)
    Pk = np.asarray(Pk_b3n, np.float32)
    W = np.asarray(W_conv, np.float32)
    gam = (np.ones(C_OUT, np.float32) if gn_gamma is None
           else np.asarray(gn_gamma, np.float32).reshape(C_OUT))
    bet = (np.zeros(C_OUT, np.float32) if gn_beta is None
           else np.asarray(gn_beta, np.float32).reshape(C_OUT))
    assert Fq.shape == (B, C_IN, N_KEYS)

    neg = bool((gam < 0).any())
    runner = _get_runner(neg)
    jax = runner.jax

    # ---- host prep: shared blobs + per-slice query blobs ----
    Fk16 = Fk.astype(np.float16)
    w1t16 = np.ascontiguousarray(W[:, :C_IN].T).astype(np.float16)
    dt16 = np.ascontiguousarray((W[:, C_IN:] - W[:, :C_IN]).T
                                ).astype(np.float16)
    sb16 = np.empty((N_CORES, 1, B16_TOT), np.float16)
    sb32 = np.empty((N_CORES, 1, B32_TOT), np.float32)
    for core in range(N_CORES):
        b, h = core // 2, core % 2
        r16 = sb16[core, 0]
        r16[FK_OFF:FK_OFF + C_IN * NKH].reshape(C_IN, NKH)[:] = \
            Fk16[b][:, h * NKH:(h + 1) * NKH]
        r16[W1_OFF:W1_OFF + C_IN * C_OUT].reshape(C_IN, C_OUT)[:] = w1t16
        r16[DT_OFF:DT_OFF + C_IN * C_OUT].reshape(C_IN, C_OUT)[:] = dt16
        pt = sb32[core, 0].reshape(4, N_KEYS)
        pt[:3] = Pk[b]
        pt[3] = -(Pk[b] * Pk[b]).sum(0)

    dev_shared = jax.device_put([sb16, sb32], [runner.shd] * 2)

    Fq16 = Fq.astype(np.float16)
    slice_outs = []
    for s in range(N_SLICES):
        fqh = np.empty((N_CORES * C_IN, NQ_S), np.float16)
        qth = np.empty((N_CORES * 4, NQ_S), np.float32)
        for core in range(N_CORES):
            b, h = core // 2, core % 2
            q0 = h * NQ + s * NQ_S
            fqh[core * C_IN:(core + 1) * C_IN] = \
                Fq16[b][:, q0:q0 + NQ_S]
            qt = qth[core * 4:(core + 1) * 4]
            qt[:3] = 2.0 * Pq[b][:, q0:q0 + NQ_S]
            qt[3] = 1.0
        dev_sl = jax.device_put([fqh, qth], [runner.shd] * 2)
        by_name = {"sb16": dev_shared[0], "sb32": dev_shared[1],
                   "fqh": dev_sl[0], "qth": dev_sl[1]}
        dev_in = [by_name[n] for n in runner.in_names]
        outs = runner.launch(dev_in, stage=s)
        # start streaming results to the host as soon as each slice lands
        for a in outs:
            for sh in a.addressable_shards:
                sh.data.copy_to_host_async()
        slice_outs.append(outs)

    # ---- drain: fetch per-slice m (+ packed stats), finish GN on host ----
    m_name_i = runner.out_names.index("m16")
    mn_name_i = runner.out_names.index("mn16") if neg else None
    st_sum = np.zeros((B, C_OUT, 2), np.float32)
    m_host = []
    for s in range(N_SLICES):
        outs = slice_outs[s]
        mfull = np.asarray(outs[m_name_i]).reshape(N_CORES, C_OUT, NQ_S + 4)
        m = mfull[:, :, :NQ_S]
        st = np.ascontiguousarray(mfull[:, :, NQ_S:]).view(np.float32)
        mn = (np.asarray(outs[mn_name_i]).reshape(N_CORES, C_OUT, NQ_S)
              if neg else None)
        m_host.append((m, mn))
        st_sum += st.reshape(B, 2, C_OUT, 2).sum(axis=1)

    # GroupNorm affine from summed stats (per batch, per group)
    cnt = float(N_KEYS * KNN * (C_OUT // G))
    sg = st_sum.reshape(B, G, C_OUT // G, 2).sum(axis=2)  # [B, G, 2]
    mean = sg[..., 0] / cnt
    var = sg[..., 1] / cnt - mean * mean
    rstd = 1.0 / np.sqrt(var + GN_EPS)
    mean_c = np.repeat(mean, C_OUT // G, axis=1)  # [B, C_OUT]
    rstd_c = np.repeat(rstd, C_OUT // G, axis=1)
    s_bc = gam[None, :] * rstd_c
    t_bc = bet[None, :] - mean_c * s_bc

    out = np.empty((B, C_OUT, N_KEYS), np.float32)
    for s in range(N_SLICES):
        m, mn = m_host[s]
        for core in range(N_CORES):
            b, h = core // 2, core % 2
            q0 = h * NQ + s * NQ_S
            v = m[core] * s_bc[b][:, None] + t_bc[b][:, None]
            if neg:
                v = np.maximum(v, mn[core] * s_bc[b][:, None]
                               + t_bc[b][:, None])
            np.maximum(v, 0.0, out=out[b, :, q0:q0 + NQ_S])
    return out


if __name__ == "__main__":
    rng = np.random.default_rng(0)
    inputs = {
        "Fq_bcn": rng.standard_normal((B, C_IN, N_KEYS)).astype(np.float32),
        "Fk_bcn": rng.standard_normal((B, C_IN, N_KEYS)).astype(np.float32),
        "Pq_b3n": rng.standard_normal((B, 3, N_KEYS)).astype(np.float32),
        "Pk_b3n": rng.standard_normal((B, 3, N_KEYS)).astype(np.float32),
        "W_conv": (rng.standard_normal((C_OUT, 2 * C_IN)).astype(np.float32)
                   / np.sqrt(2 * C_IN)),
        "gn_gamma": np.ones(C_OUT, np.float32),
        "gn_beta": np.zeros(C_OUT, np.float32),
        "k": 16,
    }
    import time
    out = kernel(**inputs)
    print("kernel out", out.shape, out.dtype, float(np.abs(out).mean()))
    for i in range(5):
        t0 = time.perf_counter()
        out = kernel(**inputs)
        t1 = time.perf_counter()
        print(f"warm call {i}: {(t1 - t0) * 1e3:.1f} ms")
